# revision 42
# baseline (speedup 1.0000x reference)
"""Multi-head attention (B=2, D=1024, L=2048, H=16) on 8 TRN2 NeuronCores.

Sharding (tensor parallel over heads, per the row-parallel-Wo recipe):
core c handles batch c//4 and head quad c%4 (4 of 16 heads).  Each core
computes Q/K/V projections only for its 4 heads (weight slices are fed
per-core), attention for those heads over the full 2048x2048 score
matrix, and the row-parallel partial output projection
Wo[:, hslice] @ C_local.  The 4 partial outputs per batch are summed on
the host during unsharding (the "all-reduce after W_O" step); no
on-device collectives -- measured AllGather/barrier costs on this stack
(~50us/MB + 35-100us entry skew) dwarf the compute they would save.

Per-core schedule (single NEFF, all engines pipelined):
  - The scalar engine is the critical resource (~16.8M exp elements);
    scores flow through kt-aligned [128, 1024] PSUM tiles (2 banks,
    double-buffered = 4 banks) so exp runs back-to-back; A@V matmuls
    are emitted one key-tile late so the next tile's score matmuls sit
    ahead of them in the PE queue (no head-of-line blocking on exp).
  - All other PE work (K/V production, Q proj, partial-Wo proj, norm
    broadcast) uses a separate 2-bank PSUM pool and is chopped into
    ~0.5-1us items popped one per exp-group boundary, with a countdown
    so items never wait on in-flight DVE chains; this keeps the PE
    stream dense (no >3.4us idle gaps that would re-throttle the HAM
    clock gate to 1.2 GHz).
  - Dual-head score matmuls use disjoint PE row groups (partition bases
    0/64) so the K=64 contractions run concurrently.
  - K/V are produced in 256-key blocks interleaved into the first
    attention pass, so exp starts ~6us in instead of after all
    projections.
  - V^T carries a ones column per head, so A@V also emits the softmax
    denominator row; normalization is reciprocal_approx_fast + a bf16
    selector-matmul broadcast (single pass, vs 2-pass fp32) + DVE
    multiply per (query-chunk, head-pair).

All matmuls bf16 (f32 PSUM); softmax stats f32.
"""

import sys
import types

import numpy as np
import ml_dtypes


def _install_axon_hooks_shim():
    """antenv.axon_hooks is absent in this image; concourse imports it when
    tracing is requested (e.g. via the BASS_TRACE env var).  Provide the
    module and, if possible, the real NTFF profiling hook so tracing works
    instead of crashing."""
    try:
        import antenv.axon_hooks  # noqa: F401
        return
    except ImportError:
        pass
    try:
        import antenv
    except ImportError:
        return
    mod = types.ModuleType("antenv.axon_hooks")
    mod._hook = None
    mod.set_axon_ntff_profile_hook = lambda h: setattr(mod, "_hook", h)
    mod.get_axon_ntff_profile_hook = lambda: mod._hook
    sys.modules["antenv.axon_hooks"] = mod
    antenv.axon_hooks = mod
    try:
        from trn_agent_boot.trn_boot import _ntff_profile_via_ctypes

        h = _ntff_profile_via_ctypes("/opt/axon/libaxon_pjrt.so")
        if h is not None:
            mod._hook = h
    except Exception:
        pass


_install_axon_hooks_shim()

import concourse.bass as bass
import concourse.mybir as mybir
import concourse.tile as tile
from concourse import bacc
from concourse.bass_utils import run_bass_kernel_spmd
from concourse.tile_rust import add_dep_helper

BF16 = mybir.dt.bfloat16
F32 = mybir.dt.float32
FP8 = mybir.dt.float8e4
AF = mybir.ActivationFunctionType

B, D, L, H = 2, 1024, 2048, 16
DH = D // H            # 64
P = 128
SCALE = 1.0 / np.sqrt(np.float32(DH))

DC = D // P            # 8 contraction chunks of the inner dim
HC = 4                 # heads per core
NP = HC // 2           # head pairs per core (2)
LT = L // P            # 16 key tiles of 128
KB = L // 256          # 8 key production blocks of 256
QC = L // 512          # 4 query chunks of 512
HV = DH + 1            # V^T per-head width incl. ones column
HVP = 80               # padded per-head V^T stride (16B-aligned for fp8 DoubleRow)
KTP = LT // 2          # 8 key-tile pairs (DoubleRow contracts 256 keys/MM)


def build():
    nc = bacc.Bacc(None, target_bir_lowering=False, debug=False)

    # All DRAM layouts are partition-major (row = (p, chunk)) so every
    # DMA line is a long contiguous run per partition -- descriptor
    # generation cost was the startup bottleneck with d-major layouts.
    xq = nc.dram_tensor("xq", [QC * P * DC, 512], BF16, kind="ExternalInput")
    wqs = nc.dram_tensor("wqs", [P * DC, 2 * P], BF16, kind="ExternalInput")
    wks = nc.dram_tensor("wks", [P * DC, 2 * P], BF16, kind="ExternalInput")
    wvs = nc.dram_tensor("wvs", [P * DC, 2 * P], BF16, kind="ExternalInput")
    wos = nc.dram_tensor("wos", [P * 2, D], BF16, kind="ExternalInput")
    selp = nc.dram_tensor("selp", [33, P], BF16, kind="ExternalInput")
    out = nc.dram_tensor("out", [P * DC, 2 * L], BF16, kind="ExternalOutput")

    xr = xq[:].rearrange("(q p o) c -> q p o c", p=P, o=DC)  # (4, 128, 8, 512)
    wqr = wqs[:].rearrange("(p ko) o -> p ko o", p=P)        # (128, 8, 256)
    wkr = wks[:].rearrange("(p ko) o -> p ko o", p=P)
    wvr = wvs[:].rearrange("(p ko) o -> p ko o", p=P)
    wor = wos[:].rearrange("(p pc) o -> p pc o", p=P)        # (128, 2, 1024)
    outr = out[:].rearrange("(p o) (pc l) -> p o pc l", p=P, pc=2)

    with tile.TileContext(nc) as tc:
        with (
            tc.tile_pool(name="consts", bufs=1) as consts,
            tc.tile_pool(name="resident", bufs=1) as res,
            tc.tile_pool(name="exp", bufs=6) as epool,
            tc.tile_pool(name="norm", bufs=2) as npool,
            tc.tile_pool(name="outp", bufs=8) as opool,
            tc.tile_pool(name="ps_sc", bufs=2, space="PSUM") as ps_sc,
            tc.tile_pool(name="ps_c", bufs=2, space="PSUM") as ps_c,
            tc.tile_pool(name="ps_x", bufs=2, space="PSUM") as ps_x,
        ):
            # ---- resident SBUF ----
            x_sb = res.tile([P, QC, DC, 512], BF16)  # x[b], quarter-major
            wq_sb = res.tile([P, DC, 2 * P], BF16)  # WqT[:, hslice]
            wk_sb = res.tile([P, DC, 2 * P], BF16)
            wv_sb = res.tile([P, DC, 2 * P], BF16)
            wo_sb = res.tile([P, 2, D], BF16)       # WoT[hslice, :]
            k_sb = res.tile([P, NP, L], BF16)       # K rows (pair-major)
            q_sb = res.tile([P, NP, L], BF16)       # Q rows (pair-major)
            vt_sb = res.tile([P, LT, HC * HV], BF16)  # V^T tiles + ones cols
            c_sb = res.tile([P, NP, L], F32)        # unnormalized C
            cn_sb = res.tile([P, NP, L], BF16)      # normalized C
            selp_sb = consts.tile([33, P], BF16)
            # denominator staging: rows 0 and 32 (32-aligned partition
            # offsets are engine-writable); rows 1-31 stay 1.0 and meet a
            # zero selector row in the broadcast matmul.
            den_all = res.tile([33, 512], F32)

            vt4 = vt_sb[:].rearrange("p l (h e) -> p l h e", e=HV)
            nc.vector.memset(vt4[:, :, :, DH : DH + 1], 1.0)
            nc.vector.memset(den_all[:], 1.0)
            jnk = consts.tile([P, 512], BF16)
            nc.vector.memset(jnk[:], 1.0)
            jnk_out = consts.tile([P, 512], F32)
            wu_ps = ps_x.tile([P, 512], F32, tag="x", name="wu")
            for _ in range(30):
                nc.tensor.matmul(
                    wu_ps[:], lhsT=jnk[:, 0:P], rhs=jnk[:], start=True,
                    stop=True,
                )
            nc.vector.tensor_copy(out=jnk_out[:], in_=wu_ps[:])

            # ---- input DMA: x quarters on the sync queue, weights on the
            # gpsimd queue so they stream in parallel ----
            # quarter-major x: both DMA sides are contiguous 8KB runs
            # per partition; selp (tiny, needed late) queues after it.
            nc.sync.dma_start(out=x_sb[:, 0], in_=xr[0])
            nc.sync.dma_start(out=selp_sb[:], in_=selp[:])
            nc.sync.dma_start(out=wq_sb[:], in_=wqr)
            for qi in range(1, QC):
                nc.sync.dma_start(out=x_sb[:, qi], in_=xr[qi])
            nc.scalar.dma_start(out=wk_sb[:], in_=wkr)
            nc.scalar.dma_start(out=wv_sb[:], in_=wvr)
            nc.gpsimd.dma_start(out=wo_sb[:], in_=wor)

            # ---------------- emission helpers ----------------
            state = {
                "tile": None,     # current [P, 1024] score psum tile
                "e": None,        # matching SBUF exp output tile
                "nslot": 0,
                "meta": [],       # (slot, kt, hip)
                "pending_av": [],  # [(e_tile, metas, cps, pair)] awaiting
                "cps": None,      # {hip: psum C accumulator} for current pair
                "pair": 0,
            }
            # misc-item scheduler: strict FIFO of (countdown, emit_fn); one
            # item considered per group boundary; countdown lets items that
            # wait on DVE chains (norm broadcast) sit a few groups first.
            items = []

            def pop_item():
                for _ in range(state.get("npop", 1)):
                    if not items:
                        return
                    cd, fn = items[0]
                    if cd > 0:
                        items[0] = (cd - 1, fn)
                        return
                    items.pop(0)
                    fn()

            def score_slot(kt):
                if state["tile"] is None:
                    state["tile"] = ps_sc.tile([P, 1024], F32, tag="sc",
                                               name="sgrp")
                    state["e"] = epool.tile([P, 1024], BF16, tag="e",
                                            name="egrp")
                    state["nslot"] = 0
                    state["meta"] = []
                j = state["nslot"]
                return state["tile"][:, j * 512 : (j + 1) * 512]

            def note_slot(kt, hip):
                state["meta"].append((state["nslot"], kt, hip))
                state["nslot"] += 1

            def emit_pending_av():
                for e_t, metas, cps, pair in state["pending_av"]:
                    for (j, kt, hip) in metas:
                        nc.tensor.matmul(
                            cps[hip][0:HV, :],
                            lhsT=vt_sb[:, kt, (2 * pair + hip) * HV
                                       : (2 * pair + hip + 1) * HV],
                            rhs=e_t[:, j * 512 : (j + 1) * 512],
                            start=(kt == 0),
                            stop=(kt == LT - 1),
                        )
                state["pending_av"] = []

            def flush_group():
                if state["tile"] is None or state["nslot"] == 0:
                    return
                n = state["nslot"]
                e_t = state["e"]
                nc.scalar.activation(
                    e_t[:, 0 : n * 512],
                    state["tile"][:, 0 : n * 512],
                    AF.Exp,
                    scale=float(SCALE),
                )
                metas = state["meta"]
                state["tile"] = None
                state["e"] = None
                while len(state["pending_av"]) >= 2:
                    oldest = state["pending_av"].pop(0)
                    saved = state["pending_av"]
                    state["pending_av"] = [oldest]
                    emit_pending_av()
                    state["pending_av"] = saved
                state["pending_av"].append((e_t, metas, state["cps"],
                                            state["pair"]))
                pop_item()

            # K/V production items (streamed through the pop queue
            # during the first attention pass, deadline-ordered).
            def produce_k_item(kb, pair):
                def emit():
                    ps = ps_x.tile([P, 512], F32, tag="x", name="pk")
                    for kt in range(DC):
                        nc.tensor.matmul(
                            ps[:],
                            lhsT=wk_sb[:, kt, pair * P : (pair + 1) * P],
                            rhs=x_sb[:, kb, kt, :],
                            start=(kt == 0),
                            stop=(kt == DC - 1),
                        )
                    nc.vector.tensor_copy(
                        out=k_sb[:, pair, kb * 512 : (kb + 1) * 512], in_=ps[:]
                    )
                return emit

            def produce_v_item(lt):
                def emit():
                    ps = ps_x.tile([P, 512], F32, tag="x", name="pv")
                    for kt in range(DC):
                        nc.tensor.matmul(
                            ps[:, 0:256],
                            lhsT=x_sb[:, lt // 4, kt,
                                      (lt % 4) * P : (lt % 4) * P + P],
                            rhs=wv_sb[:, kt, :],
                            start=(kt == 0),
                            stop=(kt == DC - 1),
                        )
                    nc.vector.tensor_copy(
                        out=vt4[:, lt, :, 0:DH],
                        in_=ps[:, 0:256].rearrange("p (h e) -> p h e", e=DH),
                    )
                return emit

            def q_proj_item(qc, pair):
                def emit():
                    ps = ps_x.tile([P, 512], F32, tag="x", name="pq")
                    for kt in range(DC):
                        nc.tensor.matmul(
                            ps[:],
                            lhsT=wq_sb[:, kt, pair * P : (pair + 1) * P],
                            rhs=x_sb[:, qc, kt, :],
                            start=(kt == 0),
                            stop=(kt == DC - 1),
                        )
                    nc.vector.tensor_copy(
                        out=q_sb[:, pair, qc * 512 : (qc + 1) * 512], in_=ps[:]
                    )
                return emit

            def norm_stash(qc, pair, cps, last=False):
                """Immediate (no PE): stage denominator rows into den_all
                rows 0/32 (32-aligned offsets), reciprocal, then stash the
                C rows.  Returns bf16 recip tile for the broadcast item."""
                for hip in range(2):
                    if last:
                        nc.scalar.copy(
                            out=den_all[32 * hip : 32 * hip + 1, :],
                            in_=cps[hip][DH : DH + 1, :],
                        )
                    else:
                        nc.vector.tensor_copy(
                            out=den_all[32 * hip : 32 * hip + 1, :],
                            in_=cps[hip][DH : DH + 1, :],
                        )
                recip = npool.tile([33, 512], F32, tag="recip")
                nc.vector.reciprocal_approx_fast(recip[:], den_all[:])
                recip_bf = npool.tile([33, 512], BF16, tag="recipb")
                nc.vector.tensor_copy(out=recip_bf[:], in_=recip[:])
                for hip in range(2):
                    nc.vector.tensor_copy(
                        out=c_sb[hip * DH : (hip + 1) * DH, pair,
                                 qc * 512 : (qc + 1) * 512],
                        in_=cps[hip][0:DH, :],
                    )
                return recip_bf

            def norm_bc_item(qc, pair, recip_bf):
                def emit():
                    bc_ps = ps_x.tile([P, 512], F32, tag="x", name="bc")
                    nc.tensor.matmul(
                        bc_ps[:], lhsT=selp_sb[:], rhs=recip_bf[:],
                        start=True, stop=True,
                    )
                    nc.vector.tensor_mul(
                        out=cn_sb[:, pair, qc * 512 : (qc + 1) * 512],
                        in0=c_sb[:, pair, qc * 512 : (qc + 1) * 512],
                        in1=bc_ps[:],
                    )
                    tail = (qc == QC - 1 and pair == NP - 1)
                    for oc in range(DC):
                        items.append((0, out_proj_item(qc, oc, pair, tail)))
                return emit

            def stash_item(qc, pair, cps):
                """Priority item popped right after the pair's last A@V is
                emitted (one flush into the next pair): stash + reciprocal,
                then queue the broadcast a couple of groups out."""
                last = (qc == QC - 1 and pair == NP - 1)
                def emit():
                    recip_bf = norm_stash(qc, pair, cps, last=last)
                    items.append((2, norm_bc_item(qc, pair, recip_bf)))
                return emit

            def out_proj_item(qc, oc, pc, tail=False):
                def emit():
                    if tail and oc % 2:
                        pst = ps_sc.tile([P, 1024], F32, tag="sc", name="po2")
                        ps = pst[:, 0:512]
                    else:
                        ps = ps_x.tile([P, 512], F32, tag="x", name="po")
                    nc.tensor.matmul(
                        ps[:],
                        lhsT=wo_sb[:, pc, oc * P : (oc + 1) * P],
                        rhs=cn_sb[:, pc, qc * 512 : (qc + 1) * 512],
                        start=True,
                        stop=True,
                    )
                    o_sb = opool.tile([P, 512], BF16, tag="o")
                    # in the post-attention drain, ACT and the scalar DMA
                    # queue are idle: alternate engines so the 8 remaining
                    # chunks pipeline ~2x faster.
                    if tail and oc % 2:
                        nc.scalar.copy(out=o_sb[:], in_=ps[:])
                        dma_eng = nc.scalar
                    else:
                        nc.vector.tensor_copy(out=o_sb[:], in_=ps[:])
                        dma_eng = nc.sync
                    dma_eng.dma_start(
                        out=outr[:, oc, pc, qc * 512 : (qc + 1) * 512],
                        in_=o_sb[:],
                    )
                return emit

            # ---------------- main schedule ----------------
            # Minimal prefix: K block 0 (pair 0) + Q (qc0, pair 0); all other
            # production streams through the pop queue at 2 items/flush
            # during the first pass, so exp starts ~15us earlier.
            produce_k_item(0, 0)()
            q_proj_item(0, 0)()
            for it in [produce_v_item(0), produce_v_item(1), produce_v_item(2),
                       produce_k_item(1, 0), produce_v_item(3),
                       produce_v_item(4), produce_v_item(5), produce_v_item(6),
                       produce_k_item(2, 0), produce_v_item(7),
                       produce_v_item(8), produce_v_item(9),
                       produce_v_item(10), produce_k_item(3, 0),
                       produce_v_item(11), produce_v_item(12),
                       produce_v_item(13), produce_v_item(14),
                       produce_k_item(0, 1), produce_v_item(15),
                       produce_k_item(1, 1), produce_k_item(2, 1),
                       produce_k_item(3, 1), q_proj_item(0, 1)]:
                items.append((0, it))
            first_pass = True
            for qc in range(QC):
                for pair in range(NP):
                    state["pair"] = pair
                    state["npop"] = 2 if first_pass else 1
                    cps_a = ps_c.tile([P, 512], F32, tag="c", name="cps_a")
                    cps_b = ps_c.tile([P, 512], F32, tag="c", name="cps_b")
                    state["cps"] = {0: cps_a, 1: cps_b}
                    for kt in range(LT):
                        for hip in range(2):
                            po = hip * DH
                            s = score_slot(kt)
                            nc.tensor.matmul(
                                s,
                                lhsT=k_sb[po : po + DH, pair,
                                          kt * P : (kt + 1) * P],
                                rhs=q_sb[po : po + DH, pair,
                                         qc * 512 : (qc + 1) * 512],
                                start=True,
                                stop=True,
                            )
                            note_slot(kt, hip)
                        flush_group()
                    first_pass = False
                    items.insert(0, (1, stash_item(qc, pair, state["cps"])))
                    if pair == 0 and qc + 1 < QC:
                        for pr in range(NP):
                            items.append((0, q_proj_item(qc + 1, pr)))
            emit_pending_av()
            while items:
                cd, fn = items.pop(0)
                fn()

    if not nc.is_finalized():
        nc.finalize()
    return nc


_NC_CACHE = {}


def _get_nc():
    if "nc" not in _NC_CACHE:
        _NC_CACHE["nc"] = build()
    return _NC_CACHE["nc"]


def _run(x, Wq, Wk, Wv, Wo, trace=False):
    """x: (B, D, L) f32; W*: (D, D) f32. Returns (out, BassKernelResults)."""
    nc = _get_nc()
    bf = ml_dtypes.bfloat16
    xb = np.ascontiguousarray(x).astype(bf)                 # (B, D, L)
    wqt = np.asarray(Wq, np.float32).T.astype(bf)           # (in, out)
    wkt = np.asarray(Wk, np.float32).T.astype(bf)
    wvt = np.asarray(Wv, np.float32).T.astype(bf)
    wot = np.asarray(Wo, np.float32).T.astype(bf)           # (in, out)

    selp = np.zeros((33, P), np.float32)
    selp[0, 0:DH] = 1.0
    selp[32, DH:P] = 1.0
    selp = selp.astype(bf)

    def pmajor(a, nchunk):
        # (nchunk*128, F) row-major (chunk, p) -> (p, chunk) partition-major
        return np.ascontiguousarray(
            a.reshape(nchunk, P, a.shape[-1]).transpose(1, 0, 2)
            .reshape(nchunk * P, a.shape[-1])
        )

    in_maps = []
    for c in range(8):
        b = c // 4
        hs = slice((c % 4) * 2 * P, (c % 4 + 1) * 2 * P)
        xquads = np.stack(
            [pmajor(xb[b][:, qi * 512 : (qi + 1) * 512], DC)
             for qi in range(QC)]
        ).reshape(QC * P * DC, 512)
        in_maps.append(
            {
                "xq": np.ascontiguousarray(xquads),
                "wqs": pmajor(wqt[:, hs], DC),
                "wks": pmajor(wkt[:, hs], DC),
                "wvs": pmajor(wvt[:, hs], DC),
                "wos": pmajor(wot[hs, :], 2),
                "selp": selp,
            }
        )
    res = run_bass_kernel_spmd(nc, in_maps, core_ids=list(range(8)), trace=trace)
    out = np.zeros((B, D, L), np.float32)
    for c in range(8):
        part = res.results[c]["out"].astype(np.float32)   # (P*DC, 2*L)
        part = part.reshape(P, DC, 2, L).sum(axis=2)
        out[c // 4] += part.transpose(1, 0, 2).reshape(D, L)
    return out, res


def kernel(x, mask, Wq, Wk, Wv, Wo):
    # mask is all-ones by construction (fill: ones) -- softmax over all keys.
    out, _ = _run(x, Wq, Wk, Wv, Wo, trace=False)
    return out


# revision 44
# speedup vs baseline: 1.0002x; 1.0002x over previous
"""Multi-head attention (B=2, D=1024, L=2048, H=16) on 8 TRN2 NeuronCores.

Sharding (tensor parallel over heads, per the row-parallel-Wo recipe):
core c handles batch c//4 and head quad c%4 (4 of 16 heads).  Each core
computes Q/K/V projections only for its 4 heads (weight slices are fed
per-core), attention for those heads over the full 2048x2048 score
matrix, and the row-parallel partial output projection
Wo[:, hslice] @ C_local.  The 4 partial outputs per batch are summed on
the host during unsharding (the "all-reduce after W_O" step); no
on-device collectives -- measured AllGather/barrier costs on this stack
(~50us/MB + 35-100us entry skew) dwarf the compute they would save.

Per-core schedule (single NEFF, all engines pipelined):
  - The scalar engine is the critical resource (~16.8M exp elements);
    scores flow through kt-aligned [128, 1024] PSUM tiles (2 banks,
    double-buffered = 4 banks) so exp runs back-to-back; A@V matmuls
    are emitted one key-tile late so the next tile's score matmuls sit
    ahead of them in the PE queue (no head-of-line blocking on exp).
  - All other PE work (K/V production, Q proj, partial-Wo proj, norm
    broadcast) uses a separate 2-bank PSUM pool and is chopped into
    ~0.5-1us items popped one per exp-group boundary, with a countdown
    so items never wait on in-flight DVE chains; this keeps the PE
    stream dense (no >3.4us idle gaps that would re-throttle the HAM
    clock gate to 1.2 GHz).
  - Dual-head score matmuls use disjoint PE row groups (partition bases
    0/64) so the K=64 contractions run concurrently.
  - K/V are produced in 256-key blocks interleaved into the first
    attention pass, so exp starts ~6us in instead of after all
    projections.
  - V^T carries a ones column per head, so A@V also emits the softmax
    denominator row; normalization is reciprocal_approx_fast + a bf16
    selector-matmul broadcast (single pass, vs 2-pass fp32) + DVE
    multiply per (query-chunk, head-pair).

All matmuls bf16 (f32 PSUM); softmax stats f32.
"""

import sys
import types

import numpy as np
import ml_dtypes


def _install_axon_hooks_shim():
    """antenv.axon_hooks is absent in this image; concourse imports it when
    tracing is requested (e.g. via the BASS_TRACE env var).  Provide the
    module and, if possible, the real NTFF profiling hook so tracing works
    instead of crashing."""
    try:
        import antenv.axon_hooks  # noqa: F401
        return
    except ImportError:
        pass
    try:
        import antenv
    except ImportError:
        return
    mod = types.ModuleType("antenv.axon_hooks")
    mod._hook = None
    mod.set_axon_ntff_profile_hook = lambda h: setattr(mod, "_hook", h)
    mod.get_axon_ntff_profile_hook = lambda: mod._hook
    sys.modules["antenv.axon_hooks"] = mod
    antenv.axon_hooks = mod
    try:
        from trn_agent_boot.trn_boot import _ntff_profile_via_ctypes

        h = _ntff_profile_via_ctypes("/opt/axon/libaxon_pjrt.so")
        if h is not None:
            mod._hook = h
    except Exception:
        pass


_install_axon_hooks_shim()

import concourse.bass as bass
import concourse.mybir as mybir
import concourse.tile as tile
from concourse import bacc
from concourse.bass_utils import run_bass_kernel_spmd
from concourse.tile_rust import add_dep_helper

BF16 = mybir.dt.bfloat16
F32 = mybir.dt.float32
FP8 = mybir.dt.float8e4
AF = mybir.ActivationFunctionType

B, D, L, H = 2, 1024, 2048, 16
DH = D // H            # 64
P = 128
SCALE = 1.0 / np.sqrt(np.float32(DH))

DC = D // P            # 8 contraction chunks of the inner dim
HC = 4                 # heads per core
NP = HC // 2           # head pairs per core (2)
LT = L // P            # 16 key tiles of 128
KB = L // 256          # 8 key production blocks of 256
QC = L // 512          # 4 query chunks of 512
HV = DH + 1            # V^T per-head width incl. ones column
HVP = 80               # padded per-head V^T stride (16B-aligned for fp8 DoubleRow)
KTP = LT // 2          # 8 key-tile pairs (DoubleRow contracts 256 keys/MM)


def build():
    nc = bacc.Bacc(None, target_bir_lowering=False, debug=False)

    # All DRAM layouts are partition-major (row = (p, chunk)) so every
    # DMA line is a long contiguous run per partition -- descriptor
    # generation cost was the startup bottleneck with d-major layouts.
    xq = nc.dram_tensor("xq", [QC * P * DC, 512], BF16, kind="ExternalInput")
    wqs = nc.dram_tensor("wqs", [P * DC, 2 * P], BF16, kind="ExternalInput")
    wks = nc.dram_tensor("wks", [P * DC, 2 * P], BF16, kind="ExternalInput")
    wvs = nc.dram_tensor("wvs", [P * DC, 2 * P], BF16, kind="ExternalInput")
    wos = nc.dram_tensor("wos", [P * 2, D], BF16, kind="ExternalInput")
    selp = nc.dram_tensor("selp", [33, P], BF16, kind="ExternalInput")
    out = nc.dram_tensor("out", [P * DC, 2 * L], BF16, kind="ExternalOutput")

    xr = xq[:].rearrange("(q p o) c -> q p o c", p=P, o=DC)  # (4, 128, 8, 512)
    wqr = wqs[:].rearrange("(p ko) o -> p ko o", p=P)        # (128, 8, 256)
    wkr = wks[:].rearrange("(p ko) o -> p ko o", p=P)
    wvr = wvs[:].rearrange("(p ko) o -> p ko o", p=P)
    wor = wos[:].rearrange("(p pc) o -> p pc o", p=P)        # (128, 2, 1024)
    outr = out[:].rearrange("(p o) (pc l) -> p o pc l", p=P, pc=2)

    with tile.TileContext(nc) as tc:
        with (
            tc.tile_pool(name="consts", bufs=1) as consts,
            tc.tile_pool(name="resident", bufs=1) as res,
            tc.tile_pool(name="exp", bufs=4) as epool,
            tc.tile_pool(name="norm", bufs=2) as npool,
            tc.tile_pool(name="outp", bufs=16) as opool,
            tc.tile_pool(name="ps_sc", bufs=2, space="PSUM") as ps_sc,
            tc.tile_pool(name="ps_c", bufs=2, space="PSUM") as ps_c,
            tc.tile_pool(name="ps_x", bufs=2, space="PSUM") as ps_x,
        ):
            # ---- resident SBUF ----
            x_sb = res.tile([P, QC, DC, 512], BF16)  # x[b], quarter-major
            wq_sb = res.tile([P, DC, 2 * P], BF16)  # WqT[:, hslice]
            wk_sb = res.tile([P, DC, 2 * P], BF16)
            wv_sb = res.tile([P, DC, 2 * P], BF16)
            wo_sb = res.tile([P, 2, D], BF16)       # WoT[hslice, :]
            k_sb = res.tile([P, NP, L], BF16)       # K rows (pair-major)
            q_sb = res.tile([P, NP, L], BF16)       # Q rows (pair-major)
            vt_sb = res.tile([P, LT, HC * HV], BF16)  # V^T tiles + ones cols
            c_sb = res.tile([P, NP, L], F32)        # unnormalized C
            cn_sb = res.tile([P, NP, L], BF16)      # normalized C
            selp_sb = consts.tile([33, P], BF16)
            # denominator staging: rows 0 and 32 (32-aligned partition
            # offsets are engine-writable); rows 1-31 stay 1.0 and meet a
            # zero selector row in the broadcast matmul.
            den_all = res.tile([33, 512], F32)

            vt4 = vt_sb[:].rearrange("p l (h e) -> p l h e", e=HV)
            nc.vector.memset(vt4[:, :, :, DH : DH + 1], 1.0)
            nc.vector.memset(den_all[:], 1.0)
            jnk = consts.tile([P, 512], BF16)
            nc.vector.memset(jnk[:], 1.0)
            jnk_out = consts.tile([P, 512], F32)
            wu_ps = ps_x.tile([P, 512], F32, tag="x", name="wu")
            for _ in range(30):
                nc.tensor.matmul(
                    wu_ps[:], lhsT=jnk[:, 0:P], rhs=jnk[:], start=True,
                    stop=True,
                )
            nc.vector.tensor_copy(out=jnk_out[:], in_=wu_ps[:])

            # ---- input DMA: x quarters on the sync queue, weights on the
            # gpsimd queue so they stream in parallel ----
            # quarter-major x: both DMA sides are contiguous 8KB runs
            # per partition; selp (tiny, needed late) queues after it.
            nc.sync.dma_start(out=x_sb[:, 0], in_=xr[0])
            nc.sync.dma_start(out=selp_sb[:], in_=selp[:])
            nc.sync.dma_start(out=wq_sb[:], in_=wqr)
            for qi in range(1, QC):
                nc.sync.dma_start(out=x_sb[:, qi], in_=xr[qi])
            nc.scalar.dma_start(out=wk_sb[:], in_=wkr)
            nc.scalar.dma_start(out=wv_sb[:], in_=wvr)
            nc.gpsimd.dma_start(out=wo_sb[:], in_=wor)

            # ---------------- emission helpers ----------------
            state = {
                "tile": None,     # current [P, 1024] score psum tile
                "e": None,        # matching SBUF exp output tile
                "nslot": 0,
                "meta": [],       # (slot, kt, hip)
                "pending_av": [],  # [(e_tile, metas, cps, pair)] awaiting
                "cps": None,      # {hip: psum C accumulator} for current pair
                "pair": 0,
            }
            # misc-item scheduler: strict FIFO of (countdown, emit_fn); one
            # item considered per group boundary; countdown lets items that
            # wait on DVE chains (norm broadcast) sit a few groups first.
            items = []

            def pop_item():
                for _ in range(state.get("npop", 1)):
                    if not items:
                        return
                    cd, fn = items[0]
                    if cd > 0:
                        items[0] = (cd - 1, fn)
                        return
                    items.pop(0)
                    fn()

            def score_slot(kt):
                if state["tile"] is None:
                    state["tile"] = ps_sc.tile([P, 1024], F32, tag="sc",
                                               name="sgrp")
                    state["e"] = epool.tile([P, 1024], BF16, tag="e",
                                            name="egrp")
                    state["nslot"] = 0
                    state["meta"] = []
                j = state["nslot"]
                return state["tile"][:, j * 512 : (j + 1) * 512]

            def note_slot(kt, hip):
                state["meta"].append((state["nslot"], kt, hip))
                state["nslot"] += 1

            def emit_pending_av():
                for e_t, metas, cps, pair in state["pending_av"]:
                    for (j, kt, hip) in metas:
                        nc.tensor.matmul(
                            cps[hip][0:HV, :],
                            lhsT=vt_sb[:, kt, (2 * pair + hip) * HV
                                       : (2 * pair + hip + 1) * HV],
                            rhs=e_t[:, j * 512 : (j + 1) * 512],
                            start=(kt == 0),
                            stop=(kt == LT - 1),
                        )
                state["pending_av"] = []

            def flush_group():
                if state["tile"] is None or state["nslot"] == 0:
                    return
                n = state["nslot"]
                e_t = state["e"]
                nc.scalar.activation(
                    e_t[:, 0 : n * 512],
                    state["tile"][:, 0 : n * 512],
                    AF.Exp,
                    scale=float(SCALE),
                )
                metas = state["meta"]
                state["tile"] = None
                state["e"] = None
                while len(state["pending_av"]) >= 2:
                    oldest = state["pending_av"].pop(0)
                    saved = state["pending_av"]
                    state["pending_av"] = [oldest]
                    emit_pending_av()
                    state["pending_av"] = saved
                state["pending_av"].append((e_t, metas, state["cps"],
                                            state["pair"]))
                pop_item()

            # K/V production items (streamed through the pop queue
            # during the first attention pass, deadline-ordered).
            def produce_k_item(kb, pair):
                def emit():
                    ps = ps_x.tile([P, 512], F32, tag="x", name="pk")
                    for kt in range(DC):
                        nc.tensor.matmul(
                            ps[:],
                            lhsT=wk_sb[:, kt, pair * P : (pair + 1) * P],
                            rhs=x_sb[:, kb, kt, :],
                            start=(kt == 0),
                            stop=(kt == DC - 1),
                        )
                    nc.vector.tensor_copy(
                        out=k_sb[:, pair, kb * 512 : (kb + 1) * 512], in_=ps[:]
                    )
                return emit

            def produce_v_item(lt):
                def emit():
                    ps = ps_x.tile([P, 512], F32, tag="x", name="pv")
                    for kt in range(DC):
                        nc.tensor.matmul(
                            ps[:, 0:256],
                            lhsT=x_sb[:, lt // 4, kt,
                                      (lt % 4) * P : (lt % 4) * P + P],
                            rhs=wv_sb[:, kt, :],
                            start=(kt == 0),
                            stop=(kt == DC - 1),
                        )
                    nc.vector.tensor_copy(
                        out=vt4[:, lt, :, 0:DH],
                        in_=ps[:, 0:256].rearrange("p (h e) -> p h e", e=DH),
                    )
                return emit

            def q_proj_item(qc, pair):
                def emit():
                    ps = ps_x.tile([P, 512], F32, tag="x", name="pq")
                    for kt in range(DC):
                        nc.tensor.matmul(
                            ps[:],
                            lhsT=wq_sb[:, kt, pair * P : (pair + 1) * P],
                            rhs=x_sb[:, qc, kt, :],
                            start=(kt == 0),
                            stop=(kt == DC - 1),
                        )
                    nc.vector.tensor_copy(
                        out=q_sb[:, pair, qc * 512 : (qc + 1) * 512], in_=ps[:]
                    )
                return emit

            def norm_stash(qc, pair, cps, last=False):
                """Immediate (no PE): stage denominator rows into den_all
                rows 0/32 (32-aligned offsets), reciprocal, then stash the
                C rows.  Returns bf16 recip tile for the broadcast item."""
                for hip in range(2):
                    if last:
                        nc.scalar.copy(
                            out=den_all[32 * hip : 32 * hip + 1, :],
                            in_=cps[hip][DH : DH + 1, :],
                        )
                    else:
                        nc.vector.tensor_copy(
                            out=den_all[32 * hip : 32 * hip + 1, :],
                            in_=cps[hip][DH : DH + 1, :],
                        )
                recip = npool.tile([33, 512], F32, tag="recip")
                nc.vector.reciprocal_approx_fast(recip[:], den_all[:])
                recip_bf = npool.tile([33, 512], BF16, tag="recipb")
                nc.vector.tensor_copy(out=recip_bf[:], in_=recip[:])
                for hip in range(2):
                    nc.vector.tensor_copy(
                        out=c_sb[hip * DH : (hip + 1) * DH, pair,
                                 qc * 512 : (qc + 1) * 512],
                        in_=cps[hip][0:DH, :],
                    )
                return recip_bf

            def norm_bc_item(qc, pair, recip_bf):
                def emit():
                    bc_ps = ps_x.tile([P, 512], F32, tag="x", name="bc")
                    nc.tensor.matmul(
                        bc_ps[:], lhsT=selp_sb[:], rhs=recip_bf[:],
                        start=True, stop=True,
                    )
                    nc.vector.tensor_mul(
                        out=cn_sb[:, pair, qc * 512 : (qc + 1) * 512],
                        in0=c_sb[:, pair, qc * 512 : (qc + 1) * 512],
                        in1=bc_ps[:],
                    )
                    tail = (qc == QC - 1 and pair == NP - 1)
                    for oc in range(DC):
                        items.append((0, out_proj_item(qc, oc, pair, tail)))
                return emit

            def stash_item(qc, pair, cps):
                """Priority item popped right after the pair's last A@V is
                emitted (one flush into the next pair): stash + reciprocal,
                then queue the broadcast a couple of groups out."""
                last = (qc == QC - 1 and pair == NP - 1)
                def emit():
                    recip_bf = norm_stash(qc, pair, cps, last=last)
                    items.append((2, norm_bc_item(qc, pair, recip_bf)))
                return emit

            def out_proj_item(qc, oc, pc, tail=False):
                def emit():
                    if tail and oc % 2:
                        pst = ps_sc.tile([P, 1024], F32, tag="sc", name="po2")
                        ps = pst[:, 0:512]
                    else:
                        ps = ps_x.tile([P, 512], F32, tag="x", name="po")
                    nc.tensor.matmul(
                        ps[:],
                        lhsT=wo_sb[:, pc, oc * P : (oc + 1) * P],
                        rhs=cn_sb[:, pc, qc * 512 : (qc + 1) * 512],
                        start=True,
                        stop=True,
                    )
                    o_sb = opool.tile([P, 512], BF16, tag="o")
                    # in the post-attention drain, ACT and the scalar DMA
                    # queue are idle: alternate engines so the 8 remaining
                    # chunks pipeline ~2x faster.
                    if tail and oc % 2:
                        nc.scalar.copy(out=o_sb[:], in_=ps[:])
                        dma_eng = nc.scalar
                    else:
                        nc.vector.tensor_copy(out=o_sb[:], in_=ps[:])
                        dma_eng = nc.sync
                    dma_eng.dma_start(
                        out=outr[:, oc, pc, qc * 512 : (qc + 1) * 512],
                        in_=o_sb[:],
                    )
                return emit

            # ---------------- main schedule ----------------
            # Minimal prefix: K block 0 (pair 0) + Q (qc0, pair 0); all other
            # production streams through the pop queue at 2 items/flush
            # during the first pass, so exp starts ~15us earlier.
            produce_k_item(0, 0)()
            q_proj_item(0, 0)()
            for it in [produce_v_item(0), produce_v_item(1), produce_v_item(2),
                       produce_k_item(1, 0), produce_v_item(3),
                       produce_v_item(4), produce_v_item(5), produce_v_item(6),
                       produce_k_item(2, 0), produce_v_item(7),
                       produce_v_item(8), produce_v_item(9),
                       produce_v_item(10), produce_k_item(3, 0),
                       produce_v_item(11), produce_v_item(12),
                       produce_v_item(13), produce_v_item(14),
                       produce_k_item(0, 1), produce_v_item(15),
                       produce_k_item(1, 1), produce_k_item(2, 1),
                       produce_k_item(3, 1), q_proj_item(0, 1)]:
                items.append((0, it))
            first_pass = True
            for qc in range(QC):
                for pair in range(NP):
                    state["pair"] = pair
                    state["npop"] = 2 if first_pass else 1
                    cps_a = ps_c.tile([P, 512], F32, tag="c", name="cps_a")
                    cps_b = ps_c.tile([P, 512], F32, tag="c", name="cps_b")
                    state["cps"] = {0: cps_a, 1: cps_b}
                    for kt in range(LT):
                        for hip in range(2):
                            po = hip * DH
                            s = score_slot(kt)
                            nc.tensor.matmul(
                                s,
                                lhsT=k_sb[po : po + DH, pair,
                                          kt * P : (kt + 1) * P],
                                rhs=q_sb[po : po + DH, pair,
                                         qc * 512 : (qc + 1) * 512],
                                start=True,
                                stop=True,
                            )
                            note_slot(kt, hip)
                        flush_group()
                    first_pass = False
                    items.insert(0, (1, stash_item(qc, pair, state["cps"])))
                    if pair == 0 and qc + 1 < QC:
                        for pr in range(NP):
                            items.append((0, q_proj_item(qc + 1, pr)))
            emit_pending_av()
            while items:
                cd, fn = items.pop(0)
                fn()

    if not nc.is_finalized():
        nc.finalize()
    return nc


_NC_CACHE = {}


def _get_nc():
    if "nc" not in _NC_CACHE:
        _NC_CACHE["nc"] = build()
    return _NC_CACHE["nc"]


def _run(x, Wq, Wk, Wv, Wo, trace=False):
    """x: (B, D, L) f32; W*: (D, D) f32. Returns (out, BassKernelResults)."""
    nc = _get_nc()
    bf = ml_dtypes.bfloat16
    xb = np.ascontiguousarray(x).astype(bf)                 # (B, D, L)
    wqt = np.asarray(Wq, np.float32).T.astype(bf)           # (in, out)
    wkt = np.asarray(Wk, np.float32).T.astype(bf)
    wvt = np.asarray(Wv, np.float32).T.astype(bf)
    wot = np.asarray(Wo, np.float32).T.astype(bf)           # (in, out)

    selp = np.zeros((33, P), np.float32)
    selp[0, 0:DH] = 1.0
    selp[32, DH:P] = 1.0
    selp = selp.astype(bf)

    def pmajor(a, nchunk):
        # (nchunk*128, F) row-major (chunk, p) -> (p, chunk) partition-major
        return np.ascontiguousarray(
            a.reshape(nchunk, P, a.shape[-1]).transpose(1, 0, 2)
            .reshape(nchunk * P, a.shape[-1])
        )

    in_maps = []
    for c in range(8):
        b = c // 4
        hs = slice((c % 4) * 2 * P, (c % 4 + 1) * 2 * P)
        xquads = np.stack(
            [pmajor(xb[b][:, qi * 512 : (qi + 1) * 512], DC)
             for qi in range(QC)]
        ).reshape(QC * P * DC, 512)
        in_maps.append(
            {
                "xq": np.ascontiguousarray(xquads),
                "wqs": pmajor(wqt[:, hs], DC),
                "wks": pmajor(wkt[:, hs], DC),
                "wvs": pmajor(wvt[:, hs], DC),
                "wos": pmajor(wot[hs, :], 2),
                "selp": selp,
            }
        )
    res = run_bass_kernel_spmd(nc, in_maps, core_ids=list(range(8)), trace=trace)
    out = np.zeros((B, D, L), np.float32)
    for c in range(8):
        part = res.results[c]["out"].astype(np.float32)   # (P*DC, 2*L)
        part = part.reshape(P, DC, 2, L).sum(axis=2)
        out[c // 4] += part.transpose(1, 0, 2).reshape(D, L)
    return out, res


def kernel(x, mask, Wq, Wk, Wv, Wo):
    # mask is all-ones by construction (fill: ones) -- softmax over all keys.
    out, _ = _run(x, Wq, Wk, Wv, Wo, trace=False)
    return out


# revision 45
# speedup vs baseline: 1.0028x; 1.0026x over previous
"""Multi-head attention (B=2, D=1024, L=2048, H=16) on 8 TRN2 NeuronCores.

Sharding (tensor parallel over heads, per the row-parallel-Wo recipe):
core c handles batch c//4 and head quad c%4 (4 of 16 heads).  Each core
computes Q/K/V projections only for its 4 heads (weight slices are fed
per-core), attention for those heads over the full 2048x2048 score
matrix, and the row-parallel partial output projection
Wo[:, hslice] @ C_local.  The 4 partial outputs per batch are summed on
the host during unsharding (the "all-reduce after W_O" step); no
on-device collectives -- measured AllGather/barrier costs on this stack
(~50us/MB + 35-100us entry skew) dwarf the compute they would save.

Per-core schedule (single NEFF, all engines pipelined):
  - The scalar engine is the critical resource (~16.8M exp elements);
    scores flow through kt-aligned [128, 1024] PSUM tiles (2 banks,
    double-buffered = 4 banks) so exp runs back-to-back; A@V matmuls
    are emitted one key-tile late so the next tile's score matmuls sit
    ahead of them in the PE queue (no head-of-line blocking on exp).
  - All other PE work (K/V production, Q proj, partial-Wo proj, norm
    broadcast) uses a separate 2-bank PSUM pool and is chopped into
    ~0.5-1us items popped one per exp-group boundary, with a countdown
    so items never wait on in-flight DVE chains; this keeps the PE
    stream dense (no >3.4us idle gaps that would re-throttle the HAM
    clock gate to 1.2 GHz).
  - Dual-head score matmuls use disjoint PE row groups (partition bases
    0/64) so the K=64 contractions run concurrently.
  - K/V are produced in 256-key blocks interleaved into the first
    attention pass, so exp starts ~6us in instead of after all
    projections.
  - V^T carries a ones column per head, so A@V also emits the softmax
    denominator row; normalization is reciprocal_approx_fast + a bf16
    selector-matmul broadcast (single pass, vs 2-pass fp32) + DVE
    multiply per (query-chunk, head-pair).

All matmuls bf16 (f32 PSUM); softmax stats f32.
"""

import sys
import types

import numpy as np
import ml_dtypes


def _install_axon_hooks_shim():
    """antenv.axon_hooks is absent in this image; concourse imports it when
    tracing is requested (e.g. via the BASS_TRACE env var).  Provide the
    module and, if possible, the real NTFF profiling hook so tracing works
    instead of crashing."""
    try:
        import antenv.axon_hooks  # noqa: F401
        return
    except ImportError:
        pass
    try:
        import antenv
    except ImportError:
        return
    mod = types.ModuleType("antenv.axon_hooks")
    mod._hook = None
    mod.set_axon_ntff_profile_hook = lambda h: setattr(mod, "_hook", h)
    mod.get_axon_ntff_profile_hook = lambda: mod._hook
    sys.modules["antenv.axon_hooks"] = mod
    antenv.axon_hooks = mod
    try:
        from trn_agent_boot.trn_boot import _ntff_profile_via_ctypes

        h = _ntff_profile_via_ctypes("/opt/axon/libaxon_pjrt.so")
        if h is not None:
            mod._hook = h
    except Exception:
        pass


_install_axon_hooks_shim()

import concourse.bass as bass
import concourse.mybir as mybir
import concourse.tile as tile
from concourse import bacc
from concourse.bass_utils import run_bass_kernel_spmd
from concourse.tile_rust import add_dep_helper

BF16 = mybir.dt.bfloat16
F32 = mybir.dt.float32
FP8 = mybir.dt.float8e4
AF = mybir.ActivationFunctionType

B, D, L, H = 2, 1024, 2048, 16
DH = D // H            # 64
P = 128
SCALE = 1.0 / np.sqrt(np.float32(DH))

DC = D // P            # 8 contraction chunks of the inner dim
HC = 4                 # heads per core
NP = HC // 2           # head pairs per core (2)
LT = L // P            # 16 key tiles of 128
KB = L // 256          # 8 key production blocks of 256
QC = L // 512          # 4 query chunks of 512
HV = DH + 1            # V^T per-head width incl. ones column
HVP = 80               # padded per-head V^T stride (16B-aligned for fp8 DoubleRow)
KTP = LT // 2          # 8 key-tile pairs (DoubleRow contracts 256 keys/MM)


def build():
    nc = bacc.Bacc(None, target_bir_lowering=False, debug=False)

    # All DRAM layouts are partition-major (row = (p, chunk)) so every
    # DMA line is a long contiguous run per partition -- descriptor
    # generation cost was the startup bottleneck with d-major layouts.
    xq = nc.dram_tensor("xq", [QC * P * DC, 512], BF16, kind="ExternalInput")
    wqs = nc.dram_tensor("wqs", [P * DC, 2 * P], BF16, kind="ExternalInput")
    wks = nc.dram_tensor("wks", [P * DC, 2 * P], BF16, kind="ExternalInput")
    wvs = nc.dram_tensor("wvs", [P * DC, 2 * P], BF16, kind="ExternalInput")
    wos = nc.dram_tensor("wos", [P * 2, D], BF16, kind="ExternalInput")
    selp = nc.dram_tensor("selp", [33, P], BF16, kind="ExternalInput")
    out = nc.dram_tensor("out", [P * DC, 2 * L], BF16, kind="ExternalOutput")

    xr = xq[:].rearrange("(q p o) c -> q p o c", p=P, o=DC)  # (4, 128, 8, 512)
    wqr = wqs[:].rearrange("(p ko) o -> p ko o", p=P)        # (128, 8, 256)
    wkr = wks[:].rearrange("(p ko) o -> p ko o", p=P)
    wvr = wvs[:].rearrange("(p ko) o -> p ko o", p=P)
    wor = wos[:].rearrange("(p pc) o -> p pc o", p=P)        # (128, 2, 1024)
    outr = out[:].rearrange("(p o) (pc l) -> p o pc l", p=P, pc=2)

    with tile.TileContext(nc) as tc:
        with (
            tc.tile_pool(name="consts", bufs=1) as consts,
            tc.tile_pool(name="resident", bufs=1) as res,
            tc.tile_pool(name="exp", bufs=4) as epool,
            tc.tile_pool(name="norm", bufs=4) as npool,
            tc.tile_pool(name="outp", bufs=8) as opool,
            tc.tile_pool(name="ps_sc", bufs=2, space="PSUM") as ps_sc,
            tc.tile_pool(name="ps_c", bufs=2, space="PSUM") as ps_c,
            tc.tile_pool(name="ps_x", bufs=2, space="PSUM") as ps_x,
        ):
            # ---- resident SBUF ----
            x_sb = res.tile([P, QC, DC, 512], BF16)  # x[b], quarter-major
            wq_sb = res.tile([P, DC, 2 * P], BF16)  # WqT[:, hslice]
            wk_sb = res.tile([P, DC, 2 * P], BF16)
            wv_sb = res.tile([P, DC, 2 * P], BF16)
            wo_sb = res.tile([P, 2, D], BF16)       # WoT[hslice, :]
            k_sb = res.tile([P, NP, L], BF16)       # K rows (pair-major)
            q_sb = res.tile([P, NP, L], BF16)       # Q rows (pair-major)
            vt_sb = res.tile([P, LT, HC * HV], BF16)  # V^T tiles + ones cols
            c_sb = res.tile([P, NP, L], F32)        # unnormalized C
            cn_sb = res.tile([P, NP, L], BF16)      # normalized C
            selp_sb = consts.tile([33, P], BF16)
            # denominator staging: rows 0 and 32 (32-aligned partition
            # offsets are engine-writable); rows 1-31 stay 1.0 and meet a
            # zero selector row in the broadcast matmul.
            den_all = res.tile([33, 512], F32)

            vt4 = vt_sb[:].rearrange("p l (h e) -> p l h e", e=HV)
            nc.vector.memset(vt4[:, :, :, DH : DH + 1], 1.0)
            nc.vector.memset(den_all[:], 1.0)
            jnk = consts.tile([P, 512], BF16)
            nc.vector.memset(jnk[:], 1.0)
            jnk_out = consts.tile([P, 512], F32)
            wu_ps = ps_x.tile([P, 512], F32, tag="x", name="wu")
            for _ in range(30):
                nc.tensor.matmul(
                    wu_ps[:], lhsT=jnk[:, 0:P], rhs=jnk[:], start=True,
                    stop=True,
                )
            nc.vector.tensor_copy(out=jnk_out[:], in_=wu_ps[:])

            # ---- input DMA: x quarters on the sync queue, weights on the
            # gpsimd queue so they stream in parallel ----
            # quarter-major x: both DMA sides are contiguous 8KB runs
            # per partition; selp (tiny, needed late) queues after it.
            nc.sync.dma_start(out=x_sb[:, 0], in_=xr[0])
            nc.sync.dma_start(out=selp_sb[:], in_=selp[:])
            nc.sync.dma_start(out=wq_sb[:], in_=wqr)
            for qi in range(1, QC):
                nc.sync.dma_start(out=x_sb[:, qi], in_=xr[qi])
            nc.scalar.dma_start(out=wk_sb[:], in_=wkr)
            nc.scalar.dma_start(out=wv_sb[:], in_=wvr)
            nc.gpsimd.dma_start(out=wo_sb[:], in_=wor)

            # ---------------- emission helpers ----------------
            state = {
                "tile": None,     # current [P, 1024] score psum tile
                "e": None,        # matching SBUF exp output tile
                "nslot": 0,
                "meta": [],       # (slot, kt, hip)
                "pending_av": [],  # [(e_tile, metas, cps, pair)] awaiting
                "cps": None,      # {hip: psum C accumulator} for current pair
                "pair": 0,
            }
            # misc-item scheduler: strict FIFO of (countdown, emit_fn); one
            # item considered per group boundary; countdown lets items that
            # wait on DVE chains (norm broadcast) sit a few groups first.
            items = []

            def pop_item():
                for _ in range(state.get("npop", 1)):
                    if not items:
                        return
                    cd, fn = items[0]
                    if cd > 0:
                        items[0] = (cd - 1, fn)
                        return
                    items.pop(0)
                    fn()

            def score_slot(kt):
                if state["tile"] is None:
                    state["tile"] = ps_sc.tile([P, 1024], F32, tag="sc",
                                               name="sgrp")
                    state["e"] = epool.tile([P, 1024], BF16, tag="e",
                                            name="egrp")
                    state["nslot"] = 0
                    state["meta"] = []
                j = state["nslot"]
                return state["tile"][:, j * 512 : (j + 1) * 512]

            def note_slot(kt, hip):
                state["meta"].append((state["nslot"], kt, hip))
                state["nslot"] += 1

            def emit_pending_av():
                for e_t, metas, cps, pair in state["pending_av"]:
                    for (j, kt, hip) in metas:
                        nc.tensor.matmul(
                            cps[hip][0:HV, :],
                            lhsT=vt_sb[:, kt, (2 * pair + hip) * HV
                                       : (2 * pair + hip + 1) * HV],
                            rhs=e_t[:, j * 512 : (j + 1) * 512],
                            start=(kt == 0),
                            stop=(kt == LT - 1),
                        )
                state["pending_av"] = []

            def flush_group():
                if state["tile"] is None or state["nslot"] == 0:
                    return
                n = state["nslot"]
                e_t = state["e"]
                nc.scalar.activation(
                    e_t[:, 0 : n * 512],
                    state["tile"][:, 0 : n * 512],
                    AF.Exp,
                    scale=float(SCALE),
                )
                metas = state["meta"]
                state["tile"] = None
                state["e"] = None
                while len(state["pending_av"]) >= 2:
                    oldest = state["pending_av"].pop(0)
                    saved = state["pending_av"]
                    state["pending_av"] = [oldest]
                    emit_pending_av()
                    state["pending_av"] = saved
                state["pending_av"].append((e_t, metas, state["cps"],
                                            state["pair"]))
                pop_item()

            # K/V production items (streamed through the pop queue
            # during the first attention pass, deadline-ordered).
            def produce_k_item(kb, pair):
                def emit():
                    ps = ps_x.tile([P, 512], F32, tag="x", name="pk")
                    for kt in range(DC):
                        nc.tensor.matmul(
                            ps[:],
                            lhsT=wk_sb[:, kt, pair * P : (pair + 1) * P],
                            rhs=x_sb[:, kb, kt, :],
                            start=(kt == 0),
                            stop=(kt == DC - 1),
                        )
                    nc.vector.tensor_copy(
                        out=k_sb[:, pair, kb * 512 : (kb + 1) * 512], in_=ps[:]
                    )
                return emit

            def produce_v_item(lt):
                def emit():
                    ps = ps_x.tile([P, 512], F32, tag="x", name="pv")
                    for kt in range(DC):
                        nc.tensor.matmul(
                            ps[:, 0:256],
                            lhsT=x_sb[:, lt // 4, kt,
                                      (lt % 4) * P : (lt % 4) * P + P],
                            rhs=wv_sb[:, kt, :],
                            start=(kt == 0),
                            stop=(kt == DC - 1),
                        )
                    nc.vector.tensor_copy(
                        out=vt4[:, lt, :, 0:DH],
                        in_=ps[:, 0:256].rearrange("p (h e) -> p h e", e=DH),
                    )
                return emit

            def q_proj_item(qc, pair):
                def emit():
                    ps = ps_x.tile([P, 512], F32, tag="x", name="pq")
                    for kt in range(DC):
                        nc.tensor.matmul(
                            ps[:],
                            lhsT=wq_sb[:, kt, pair * P : (pair + 1) * P],
                            rhs=x_sb[:, qc, kt, :],
                            start=(kt == 0),
                            stop=(kt == DC - 1),
                        )
                    nc.vector.tensor_copy(
                        out=q_sb[:, pair, qc * 512 : (qc + 1) * 512], in_=ps[:]
                    )
                return emit

            def norm_stash(qc, pair, cps, last=False):
                """Immediate (no PE): stage denominator rows into den_all
                rows 0/32 (32-aligned offsets), reciprocal, then stash the
                C rows.  Returns bf16 recip tile for the broadcast item."""
                for hip in range(2):
                    if last:
                        nc.scalar.copy(
                            out=den_all[32 * hip : 32 * hip + 1, :],
                            in_=cps[hip][DH : DH + 1, :],
                        )
                    else:
                        nc.vector.tensor_copy(
                            out=den_all[32 * hip : 32 * hip + 1, :],
                            in_=cps[hip][DH : DH + 1, :],
                        )
                recip = npool.tile([33, 512], F32, tag="recip")
                nc.vector.reciprocal_approx_fast(recip[:], den_all[:])
                recip_bf = npool.tile([33, 512], BF16, tag="recipb")
                nc.vector.tensor_copy(out=recip_bf[:], in_=recip[:])
                for hip in range(2):
                    nc.vector.tensor_copy(
                        out=c_sb[hip * DH : (hip + 1) * DH, pair,
                                 qc * 512 : (qc + 1) * 512],
                        in_=cps[hip][0:DH, :],
                    )
                return recip_bf

            def norm_bc_item(qc, pair, recip_bf):
                def emit():
                    bc_ps = ps_x.tile([P, 512], F32, tag="x", name="bc")
                    nc.tensor.matmul(
                        bc_ps[:], lhsT=selp_sb[:], rhs=recip_bf[:],
                        start=True, stop=True,
                    )
                    nc.vector.tensor_mul(
                        out=cn_sb[:, pair, qc * 512 : (qc + 1) * 512],
                        in0=c_sb[:, pair, qc * 512 : (qc + 1) * 512],
                        in1=bc_ps[:],
                    )
                    tail = (qc == QC - 1 and pair == NP - 1)
                    for oc in range(DC):
                        items.append((0, out_proj_item(qc, oc, pair, tail)))
                return emit

            def stash_item(qc, pair, cps):
                """Priority item popped right after the pair's last A@V is
                emitted (one flush into the next pair): stash + reciprocal,
                then queue the broadcast a couple of groups out."""
                last = (qc == QC - 1 and pair == NP - 1)
                def emit():
                    recip_bf = norm_stash(qc, pair, cps, last=last)
                    items.append((2, norm_bc_item(qc, pair, recip_bf)))
                return emit

            def out_proj_item(qc, oc, pc, tail=False):
                def emit():
                    if tail and oc % 2:
                        pst = ps_sc.tile([P, 1024], F32, tag="sc", name="po2")
                        ps = pst[:, 0:512]
                    else:
                        ps = ps_x.tile([P, 512], F32, tag="x", name="po")
                    nc.tensor.matmul(
                        ps[:],
                        lhsT=wo_sb[:, pc, oc * P : (oc + 1) * P],
                        rhs=cn_sb[:, pc, qc * 512 : (qc + 1) * 512],
                        start=True,
                        stop=True,
                    )
                    o_sb = opool.tile([P, 512], BF16, tag="o")
                    # in the post-attention drain, ACT and the scalar DMA
                    # queue are idle: alternate engines so the 8 remaining
                    # chunks pipeline ~2x faster.
                    if tail and oc % 2:
                        nc.scalar.copy(out=o_sb[:], in_=ps[:])
                        dma_eng = nc.scalar
                    else:
                        nc.vector.tensor_copy(out=o_sb[:], in_=ps[:])
                        dma_eng = nc.sync
                    dma_eng.dma_start(
                        out=outr[:, oc, pc, qc * 512 : (qc + 1) * 512],
                        in_=o_sb[:],
                    )
                return emit

            # ---------------- main schedule ----------------
            # Minimal prefix: K block 0 (pair 0) + Q (qc0, pair 0); all other
            # production streams through the pop queue at 2 items/flush
            # during the first pass, so exp starts ~15us earlier.
            produce_k_item(0, 0)()
            q_proj_item(0, 0)()
            for it in [produce_v_item(0), produce_v_item(1), produce_v_item(2),
                       produce_k_item(1, 0), produce_v_item(3),
                       produce_v_item(4), produce_v_item(5), produce_v_item(6),
                       produce_k_item(2, 0), produce_v_item(7),
                       produce_v_item(8), produce_v_item(9),
                       produce_v_item(10), produce_k_item(3, 0),
                       produce_v_item(11), produce_v_item(12),
                       produce_v_item(13), produce_v_item(14),
                       produce_k_item(0, 1), produce_v_item(15),
                       produce_k_item(1, 1), produce_k_item(2, 1),
                       produce_k_item(3, 1), q_proj_item(0, 1)]:
                items.append((0, it))
            first_pass = True
            for qc in range(QC):
                for pair in range(NP):
                    state["pair"] = pair
                    state["npop"] = 2 if first_pass else 1
                    cps_a = ps_c.tile([P, 512], F32, tag="c", name="cps_a")
                    cps_b = ps_c.tile([P, 512], F32, tag="c", name="cps_b")
                    state["cps"] = {0: cps_a, 1: cps_b}
                    for kt in range(LT):
                        for hip in range(2):
                            po = hip * DH
                            s = score_slot(kt)
                            nc.tensor.matmul(
                                s,
                                lhsT=k_sb[po : po + DH, pair,
                                          kt * P : (kt + 1) * P],
                                rhs=q_sb[po : po + DH, pair,
                                         qc * 512 : (qc + 1) * 512],
                                start=True,
                                stop=True,
                            )
                            note_slot(kt, hip)
                        flush_group()
                    first_pass = False
                    items.insert(0, (1, stash_item(qc, pair, state["cps"])))
                    if pair == 0 and qc + 1 < QC:
                        for pr in range(NP):
                            items.append((0, q_proj_item(qc + 1, pr)))
            emit_pending_av()
            while items:
                cd, fn = items.pop(0)
                fn()

    if not nc.is_finalized():
        nc.finalize()
    return nc


_NC_CACHE = {}


def _get_nc():
    if "nc" not in _NC_CACHE:
        _NC_CACHE["nc"] = build()
    return _NC_CACHE["nc"]


def _run(x, Wq, Wk, Wv, Wo, trace=False):
    """x: (B, D, L) f32; W*: (D, D) f32. Returns (out, BassKernelResults)."""
    nc = _get_nc()
    bf = ml_dtypes.bfloat16
    xb = np.ascontiguousarray(x).astype(bf)                 # (B, D, L)
    wqt = np.asarray(Wq, np.float32).T.astype(bf)           # (in, out)
    wkt = np.asarray(Wk, np.float32).T.astype(bf)
    wvt = np.asarray(Wv, np.float32).T.astype(bf)
    wot = np.asarray(Wo, np.float32).T.astype(bf)           # (in, out)

    selp = np.zeros((33, P), np.float32)
    selp[0, 0:DH] = 1.0
    selp[32, DH:P] = 1.0
    selp = selp.astype(bf)

    def pmajor(a, nchunk):
        # (nchunk*128, F) row-major (chunk, p) -> (p, chunk) partition-major
        return np.ascontiguousarray(
            a.reshape(nchunk, P, a.shape[-1]).transpose(1, 0, 2)
            .reshape(nchunk * P, a.shape[-1])
        )

    in_maps = []
    for c in range(8):
        b = c // 4
        hs = slice((c % 4) * 2 * P, (c % 4 + 1) * 2 * P)
        xquads = np.stack(
            [pmajor(xb[b][:, qi * 512 : (qi + 1) * 512], DC)
             for qi in range(QC)]
        ).reshape(QC * P * DC, 512)
        in_maps.append(
            {
                "xq": np.ascontiguousarray(xquads),
                "wqs": pmajor(wqt[:, hs], DC),
                "wks": pmajor(wkt[:, hs], DC),
                "wvs": pmajor(wvt[:, hs], DC),
                "wos": pmajor(wot[hs, :], 2),
                "selp": selp,
            }
        )
    res = run_bass_kernel_spmd(nc, in_maps, core_ids=list(range(8)), trace=trace)
    out = np.zeros((B, D, L), np.float32)
    for c in range(8):
        part = res.results[c]["out"].astype(np.float32)   # (P*DC, 2*L)
        part = part.reshape(P, DC, 2, L).sum(axis=2)
        out[c // 4] += part.transpose(1, 0, 2).reshape(D, L)
    return out, res


def kernel(x, mask, Wq, Wk, Wv, Wo):
    # mask is all-ones by construction (fill: ones) -- softmax over all keys.
    out, _ = _run(x, Wq, Wk, Wv, Wo, trace=False)
    return out
